# revision 8
# baseline (speedup 1.0000x reference)
# Multi-head attention (B=2, S=2048, D=1024, H=16) on 8 TRN2 NeuronCores.
#
# Sharding (hardcoded): core c in [0..8) handles batch b = c//4 and head
# group g = c%4 (4 heads = 256 output features of wq/wk/wv, 256 input rows
# of wo). Each core computes a partial output projection [S, D]; the host
# sums the 4 partials per batch and adds wo_bias (row-parallel unshard).
#
# Device-side pipeline (v2):
#   - activations enter transposed ([D, S]) so every matmul contracts over
#     the partition axis with no on-device transposes;
#   - scores are computed transposed (S^T[k, q]) so softmax(P) feeds the
#     P@V matmul directly (contraction over k on partitions);
#   - softmax denominator rides as a ones-column in each head's V block;
#   - attention is ScalarE(exp)-bound: a lookahead-1 software pipeline
#     (s_ps bufs=2, o_ps bufs=2 -> all 8 PSUM banks) keeps ACT saturated;
#   - projection drains + q/k biases run on ScalarE (idle in prologue);
#   - out-projection pairs heads on partitions (K=128) at the tail.
import functools
import sys

import numpy as np

try:
    import concourse  # noqa: F401
except ImportError:  # harness env without the default path
    sys.path.insert(0, "/opt/trn_rl_repo")
    sys.path.insert(0, "/opt/pypackages")

import ml_dtypes

BF16 = ml_dtypes.bfloat16

B, S, D, H = 2, 2048, 1024, 16
HD = D // H          # 64
NCORES = 8
GH = 4               # head groups (tensor-parallel)
HPG = H // GH        # heads per group = 4
DG = D // GH         # features per group = 256
P = 128              # partitions
TDIN = D // P        # 8 din tiles
QC = 2               # q-chunks of 1024 for attention
QW = S // QC         # 1024
KT = S // P          # 16 k tiles
NT2 = DG // P        # 2 dout tiles per group
NPAIR = HPG // 2     # head pairs = 2


def build_graph():
    """Build the SPMD Bass graph (identical on all 8 cores)."""
    from contextlib import ExitStack

    from concourse import bacc, mybir, tile

    f32 = mybir.dt.float32
    bf16 = mybir.dt.bfloat16
    EXP = mybir.ActivationFunctionType.Exp
    IDENT = mybir.ActivationFunctionType.Identity
    COPY = mybir.ActivationFunctionType.Copy

    nc = bacc.Bacc(
        "TRN2", target_bir_lowering=False, debug=False, num_devices=NCORES
    )

    xq = nc.dram_tensor("xq_t", (P, TDIN, S), bf16, kind="ExternalInput")
    xk = nc.dram_tensor("xk_t", (P, TDIN, S), bf16, kind="ExternalInput")
    xv = nc.dram_tensor("xv_t", (P, TDIN, S), bf16, kind="ExternalInput")
    mk = nc.dram_tensor("mask_t", (S, S), bf16, kind="ExternalInput")
    wq = nc.dram_tensor("wq", (P, TDIN, DG), bf16, kind="ExternalInput")
    wk = nc.dram_tensor("wk", (P, TDIN, DG), bf16, kind="ExternalInput")
    wv = nc.dram_tensor("wv", (P, TDIN, DG), bf16, kind="ExternalInput")
    # wo pre-arranged host-side to [128, NPAIR, D]: pair p holds head 2p's
    # 64 rows on partitions 0..63 and head 2p+1's on partitions 64..127.
    wo = nc.dram_tensor("wo", (P, NPAIR, D), bf16, kind="ExternalInput")
    # q/k biases as [128, NT2] f32 (per-partition scalars for the drain).
    qb = nc.dram_tensor("qb", (P, NT2), f32, kind="ExternalInput")
    kb = nc.dram_tensor("kb", (P, NT2), f32, kind="ExternalInput")
    vb = nc.dram_tensor("vb", (1, DG), bf16, kind="ExternalInput")
    out = nc.dram_tensor("out", (S, D), bf16, kind="ExternalOutput")

    with tile.TileContext(nc) as tc, ExitStack() as ctx:
        wpool = ctx.enter_context(tc.tile_pool(name="wpool", bufs=1))
        cpool = ctx.enter_context(tc.tile_pool(name="cpool", bufs=1))
        qkpool = ctx.enter_context(tc.tile_pool(name="qk", bufs=1))
        vpool = ctx.enter_context(tc.tile_pool(name="vsb", bufs=1))
        mpool = ctx.enter_context(tc.tile_pool(name="msk", bufs=1))
        opool = ctx.enter_context(tc.tile_pool(name="otn", bufs=1))
        ptpool = ctx.enter_context(tc.tile_pool(name="ptile", bufs=3))
        npool = ctx.enter_context(tc.tile_pool(name="small", bufs=2))
        ospool = ctx.enter_context(tc.tile_pool(name="outsb", bufs=2))
        dpool = ctx.enter_context(tc.tile_pool(name="dscr", bufs=2, space="DRAM"))
        ps = ctx.enter_context(tc.tile_pool(name="ps", bufs=2, space="PSUM"))

        # ---- persistent SBUF tensors -------------------------------------
        wk_sb = wpool.tile([P, TDIN, DG], bf16)
        wv_sb = wpool.tile([P, TDIN, DG], bf16)
        wq_sb = wpool.tile([P, TDIN, DG], bf16)
        for wsb_, wdr_ in ((wk_sb, wk), (wv_sb, wv), (wq_sb, wq)):
            for th_ in range(2):
                nc.sync.dma_start(
                    wsb_[:, th_ * 4 : (th_ + 1) * 4, :],
                    wdr_.ap()[:, th_ * 4 : (th_ + 1) * 4, :],
                )
        qb_sb = cpool.tile([P, NT2], f32)
        kb_sb = cpool.tile([P, NT2], f32)
        vb_sb = cpool.tile([1, DG], bf16)
        nc.sync.dma_start(qb_sb[:], qb.ap())
        nc.sync.dma_start(kb_sb[:], kb.ap())
        nc.sync.dma_start(vb_sb[:], vb.ap())
        ones2 = cpool.tile([1, P], bf16)
        nc.vector.memset(ones2[:], 1.0)

        qT_sb = qkpool.tile([P, NT2, S], bf16)   # q projection, transposed
        kT_sb = qkpool.tile([P, NT2, S], bf16)
        # v blocks: per k-tile, per head: [v(64) | ones] -> 65 cols
        v_sb = vpool.tile([P, KT, HPG * (HD + 1)], bf16)
        nc.vector.memset(
            v_sb[:].rearrange("p s (h x) -> p s h x", h=HPG)[:, :, :, HD : HD + 1],
            1.0,
        )

        # ---- projections (k first, then v, then q) -----------------------
        # q, k: out qT[dout, s] = wq^T(stationary) x q^T(moving) + bias.
        # drains ride ScalarE (idle in this phase); bias folds into the
        # Identity-activation drain as a per-partition scalar.
        # x loads are whole-tensor, th-split into 4 DMAs with 8KB-contiguous
        # rows, except k whose first chunk is small for a fast PE start.
        xpool_cm = tc.tile_pool(name="xin", bufs=2)
        xpool = xpool_cm.__enter__()

        def load_x_full(xdram, nm):
            xch = xpool.tile([P, TDIN, S], bf16, tag="xch", name=nm)
            for th_ in range(4):
                nc.sync.dma_start(
                    xch[:, th_ * 2 : (th_ + 1) * 2, :],
                    xdram.ap()[:, th_ * 2 : (th_ + 1) * 2, :],
                )
            return xch

        def qk_chain(xch, wsb, bias_sb, dest, half, s0):
            for dt in range(NT2):
                pj = ps.tile([P, 512], f32, tag="s", name=f"pj_{s0}_{dt}")
                for ktl in range(TDIN):
                    nc.tensor.matmul(
                        pj[:],
                        lhsT=wsb[:, ktl, dt * P : (dt + 1) * P],
                        rhs=xch[:, ktl, half * 512 : (half + 1) * 512],
                        start=(ktl == 0),
                        stop=(ktl == TDIN - 1),
                    )
                nc.scalar.activation(
                    dest[:, dt, s0 : s0 + 512],
                    pj[:],
                    IDENT,
                    bias=bias_sb[:, dt : dt + 1],
                    scale=1.0,
                )

        # k: two 1024-col chunks (first PE work starts on 2.5MB landed)
        for sc in range(2):
            xck = xpool.tile([P, TDIN, 1024], bf16, tag="xch", name=f"xk{sc}")
            for th_ in range(4):
                nc.sync.dma_start(
                    xck[:, th_ * 2 : (th_ + 1) * 2, :],
                    xk.ap()[
                        :, th_ * 2 : (th_ + 1) * 2, sc * 1024 : (sc + 1) * 1024
                    ],
                )
            for half in range(2):
                qk_chain(xck, wk_sb, kb_sb, kT_sb, half, sc * 1024 + half * 512)

        # v: natural layout [s, dout] + bias, drained per-head with ones col
        xcv = load_x_full(xv, "xcv")
        for m in range(S // P):
            pv = ps.tile([P, DG], f32, tag="s", name=f"pv_{m}")
            for ktl in range(TDIN):
                nc.tensor.matmul(
                    pv[:],
                    lhsT=xcv[:, ktl, m * P : (m + 1) * P],
                    rhs=wv_sb[:, ktl, :],
                    start=(ktl == 0),
                    stop=False,
                )
            nc.tensor.matmul(
                pv[:],
                lhsT=ones2[0:1, :],
                rhs=vb_sb[:],
                start=False,
                stop=True,
            )
            nc.scalar.activation(
                v_sb[:, m, :].rearrange("p (h x) -> p h x", h=HPG)[:, :, 0:HD],
                pv[:].rearrange("p (h x) -> p h x", h=HPG),
                COPY,
            )

        xcq = load_x_full(xq, "xcq")
        # mask (qc0 halves first: needed right as attention starts) and wo
        # are queued behind the x loads so they don't delay projections.
        mask_sb = mpool.tile([P, KT, S], bf16)
        mk_r = mk.ap().rearrange("(t p) q -> p t q", p=P)
        for kt in range(KT):
            nc.sync.dma_start(mask_sb[:, kt, 0:QW], mk_r[:, kt, 0:QW])
        for half in range(4):
            qk_chain(xcq, wq_sb, qb_sb, qT_sb, half, half * 512)
        xpool_cm.__exit__(None, None, None)
        for kt in range(KT):
            nc.sync.dma_start(mask_sb[:, kt, QW:S], mk_r[:, kt, QW:S])
        wo_sb = wpool.tile([P, NPAIR, D], bf16)
        nc.sync.dma_start(wo_sb[:], wo.ap())

        # ---- attention ---------------------------------------------------
        # otn2: pair p holds head 2p (partitions 0..63) and head 2p+1
        # (partitions 64..127, via a partition-shift DMA after the norm).
        otn2 = opool.tile([P, NPAIR, S], bf16)

        # The softmax norm of head block i is pipelined into block i+1:
        # its reciprocal + broadcast-bounce start at the top of the next
        # block, and the final multiply lands after 2 mask-muls so the DVE
        # queue never stalls the exp->mul->PV chain.
        pending = None  # (o_ps, pair, parity, q0, tagid)

        def norm_start(pend):
            o_ps, pair, parity, q0, tagid = pend
            rec65 = npool.tile([HD + 1, QW], f32, tag="rec", name=f"rec_{tagid}")
            nc.vector.reciprocal_approx_fast(out=rec65[:], in_=o_ps[:])
            scr = dpool.tile([1, QW], f32, tag="scr", name=f"scr_{tagid}")
            nc.sync.dma_start(scr[:], rec65[HD : HD + 1, :])
            rb = npool.tile([HD, QW], f32, tag="rb", name=f"rb_{tagid}")
            nc.sync.dma_start(rb[:], scr[:].to_broadcast((HD, QW)))
            return rb

        def norm_finish(pend, rb):
            o_ps, pair, parity, q0, tagid = pend
            if parity == 0:
                nc.vector.tensor_mul(
                    otn2[0:HD, pair, q0 : q0 + QW], o_ps[0:HD, :], rb[:]
                )
            else:
                otmp = npool.tile([HD, QW], bf16, tag="otmp", name=f"ot_{tagid}")
                nc.vector.tensor_mul(otmp[:], o_ps[0:HD, :], rb[:])
                nc.sync.dma_start(otn2[HD:P, pair, q0 : q0 + QW], otmp[:])

        for qc in range(QC):
            q0 = qc * QW
            for h in (0, 1, 3, 2):  # even head last: its norm writes otn2
                t, po = h // 2, (h % 2) * HD  # directly (no shift DMA)
                pair = h // 2

                def s_mm(kt):
                    sp = ps.tile(
                        [P, QW], f32, tag="s", name=f"s_{qc}_{h}_{kt}"
                    )
                    for hf in range(2):
                        nc.tensor.matmul(
                            sp[:, hf * 512 : (hf + 1) * 512],
                            lhsT=kT_sb[po : po + HD, t, kt * P : (kt + 1) * P],
                            rhs=qT_sb[
                                po : po + HD,
                                t,
                                q0 + hf * 512 : q0 + (hf + 1) * 512,
                            ],
                            start=True,
                            stop=True,
                        )
                    return sp

                o_ps = ps.tile([HD + 1, QW], f32, tag="o", name=f"o_{qc}_{h}")
                rb = norm_start(pending) if pending is not None else None
                sps = s_mm(0)
                for kt in range(KT):
                    nxt = s_mm(kt + 1) if kt + 1 < KT else None
                    pt = ptpool.tile(
                        [P, QW], bf16, tag="pt", name=f"pt_{qc}_{h}_{kt}"
                    )
                    nc.scalar.activation(pt[:], sps[:], EXP, scale=0.125)
                    nc.vector.tensor_mul(
                        pt[:], pt[:], mask_sb[:, kt, q0 : q0 + QW]
                    )
                    if kt == 2 and pending is not None:
                        norm_finish(pending, rb)
                    for hf in range(2):
                        nc.tensor.matmul(
                            o_ps[:, hf * 512 : (hf + 1) * 512],
                            lhsT=v_sb[:, kt, h * 65 : (h + 1) * 65],
                            rhs=pt[:, hf * 512 : (hf + 1) * 512],
                            start=(kt == 0),
                            stop=(kt == KT - 1),
                        )
                    sps = nxt

                pending = (o_ps, pair, h % 2, q0, f"{qc}_{h}")

        rb = norm_start(pending)
        norm_finish(pending, rb)

        # ---- output projection (tail): pairs stacked on partitions ------
        for st in range(S // P):
            op_ps = ps.tile([P, D], f32, tag="o", name=f"op_{st}")
            for nch in range(2):
                for pr in range(NPAIR):
                    nc.tensor.matmul(
                        op_ps[:, nch * 512 : (nch + 1) * 512],
                        lhsT=otn2[:, pr, st * P : (st + 1) * P],
                        rhs=wo_sb[:, pr, nch * 512 : (nch + 1) * 512],
                        start=(pr == 0),
                        stop=(pr == NPAIR - 1),
                    )
            osb2 = ospool.tile([P, D], bf16, tag="outsb", name=f"outsb_{st}")
            if st % 2 == 0:
                nc.scalar.activation(osb2[:], op_ps[:], COPY)
            else:
                nc.vector.tensor_copy(osb2[:], op_ps[:])
            nc.sync.dma_start(out.ap()[st * P : (st + 1) * P, :], osb2[:])

    nc.compile()
    return nc


@functools.lru_cache(maxsize=1)
def _graph():
    return build_graph()


def make_in_maps(
    query, key, value, mask,
    wq_kernel, wq_bias, wk_kernel, wk_bias,
    wv_kernel, wv_bias, wo_kernel, wo_bias,
):
    q = np.asarray(query, np.float32)
    k = np.asarray(key, np.float32)
    v = np.asarray(value, np.float32)
    mask = np.asarray(mask)
    wqk = np.asarray(wq_kernel, np.float32)
    wkk = np.asarray(wk_kernel, np.float32)
    wvk = np.asarray(wv_kernel, np.float32)
    wok = np.asarray(wo_kernel, np.float32)

    def tile_x(a):  # [S, D] -> [P, TDIN, S] pre-tiled transpose
        return np.ascontiguousarray(
            a.T.reshape(TDIN, P, S).transpose(1, 0, 2)
        ).astype(BF16)

    xt = [[tile_x(x[b]) for x in (q, k, v)] for b in range(B)]
    mt = [
        np.ascontiguousarray(mask[b].T.astype(np.float32)).astype(BF16)
        for b in range(B)
    ]
    in_maps = []
    for c in range(NCORES):
        b, g = divmod(c, GH)
        cs = slice(g * DG, (g + 1) * DG)
        # wo pair layout: [128, NPAIR, D]; pair p = heads (2p, 2p+1)
        wo_arr = np.ascontiguousarray(
            wok[cs, :].reshape(NPAIR, P, D).transpose(1, 0, 2)
        ).astype(BF16)
        in_maps.append(
            {
                "xq_t": xt[b][0],
                "xk_t": xt[b][1],
                "xv_t": xt[b][2],
                "mask_t": mt[b],
                "wq": np.ascontiguousarray(wqk[:, cs].reshape(TDIN, P, DG).transpose(1, 0, 2)).astype(BF16),
                "wk": np.ascontiguousarray(wkk[:, cs].reshape(TDIN, P, DG).transpose(1, 0, 2)).astype(BF16),
                "wv": np.ascontiguousarray(wvk[:, cs].reshape(TDIN, P, DG).transpose(1, 0, 2)).astype(BF16),
                "wo": wo_arr,
                "qb": np.ascontiguousarray(
                    np.asarray(wq_bias, np.float32)[cs].reshape(NT2, P).T
                ),
                "kb": np.ascontiguousarray(
                    np.asarray(wk_bias, np.float32)[cs].reshape(NT2, P).T
                ),
                "vb": np.asarray(wv_bias, np.float32)[cs].reshape(1, DG).astype(BF16),
            }
        )
    return in_maps


def combine_outputs(results, wo_bias):
    outs = np.stack([np.asarray(r["out"], np.float32) for r in results])
    full = outs.reshape(B, GH, S, D).sum(axis=1)
    return (full + np.asarray(wo_bias, np.float32)[None, None, :]).astype(
        np.float32
    )


def kernel(**inputs):
    from concourse import bass_utils

    nc = _graph()
    in_maps = make_in_maps(**inputs)
    res = bass_utils.run_bass_kernel_spmd(
        nc, in_maps, core_ids=list(range(NCORES))
    )
    return combine_outputs(res.results, inputs["wo_bias"])


# revision 10
# speedup vs baseline: 1.2284x; 1.2284x over previous
# Multi-head attention (B=2, S=2048, D=1024, H=16) on 8 TRN2 NeuronCores.
#
# Sharding (hardcoded): core c in [0..8) handles batch b = c//4 and head
# group g = c%4 (4 heads = 256 output features of wq/wk/wv, 256 input rows
# of wo). Each core computes a partial output projection [S, D]; the host
# sums the 4 partials per batch and adds wo_bias (row-parallel unshard).
#
# Device-side pipeline (v2):
#   - activations enter transposed ([D, S]) so every matmul contracts over
#     the partition axis with no on-device transposes;
#   - scores are computed transposed (S^T[k, q]) so softmax(P) feeds the
#     P@V matmul directly (contraction over k on partitions);
#   - softmax denominator rides as a ones-column in each head's V block;
#   - attention is ScalarE(exp)-bound: a lookahead-1 software pipeline
#     (s_ps bufs=2, o_ps bufs=2 -> all 8 PSUM banks) keeps ACT saturated;
#   - projection drains + q/k biases run on ScalarE (idle in prologue);
#   - out-projection pairs heads on partitions (K=128) at the tail.
import functools
import sys

import numpy as np

try:
    import concourse  # noqa: F401
except ImportError:  # harness env without the default path
    sys.path.insert(0, "/opt/trn_rl_repo")
    sys.path.insert(0, "/opt/pypackages")

import ml_dtypes

BF16 = ml_dtypes.bfloat16

B, S, D, H = 2, 2048, 1024, 16
HD = D // H          # 64
NCORES = 8
GH = 4               # head groups (tensor-parallel)
HPG = H // GH        # heads per group = 4
DG = D // GH         # features per group = 256
P = 128              # partitions
TDIN = D // P        # 8 din tiles
QC = 2               # q-chunks of 1024 for attention
QW = S // QC         # 1024
KT = S // P          # 16 k tiles
NT2 = DG // P        # 2 dout tiles per group
NPAIR = HPG // 2     # head pairs = 2


def build_graph():
    """Build the SPMD Bass graph (identical on all 8 cores)."""
    from contextlib import ExitStack

    from concourse import bacc, mybir, tile

    f32 = mybir.dt.float32
    bf16 = mybir.dt.bfloat16
    EXP = mybir.ActivationFunctionType.Exp
    IDENT = mybir.ActivationFunctionType.Identity
    COPY = mybir.ActivationFunctionType.Copy

    nc = bacc.Bacc(
        "TRN2", target_bir_lowering=False, debug=False, num_devices=NCORES
    )

    xq = nc.dram_tensor("xq_t", (P, TDIN, S), bf16, kind="ExternalInput")
    xk = nc.dram_tensor("xk_t", (P, TDIN, S), bf16, kind="ExternalInput")
    xv = nc.dram_tensor("xv_t", (P, TDIN, S), bf16, kind="ExternalInput")
    mk = nc.dram_tensor("mask_t", (S, S), bf16, kind="ExternalInput")
    wq = nc.dram_tensor("wq", (P, TDIN, DG), bf16, kind="ExternalInput")
    wk = nc.dram_tensor("wk", (P, TDIN, DG), bf16, kind="ExternalInput")
    wv = nc.dram_tensor("wv", (P, TDIN, DG), bf16, kind="ExternalInput")
    # wo pre-arranged host-side to [128, NPAIR, D]: pair p holds head 2p's
    # 64 rows on partitions 0..63 and head 2p+1's on partitions 64..127.
    wo = nc.dram_tensor("wo", (P, NPAIR, D), bf16, kind="ExternalInput")
    # q/k biases as [128, NT2] f32 (per-partition scalars for the drain).
    qb = nc.dram_tensor("qb", (P, NT2), f32, kind="ExternalInput")
    kb = nc.dram_tensor("kb", (P, NT2), f32, kind="ExternalInput")
    vb = nc.dram_tensor("vb", (1, DG), bf16, kind="ExternalInput")
    out = nc.dram_tensor("out", (S, D), bf16, kind="ExternalOutput")

    with tile.TileContext(nc) as tc, ExitStack() as ctx:
        wpool = ctx.enter_context(tc.tile_pool(name="wpool", bufs=1))
        cpool = ctx.enter_context(tc.tile_pool(name="cpool", bufs=1))
        qkpool = ctx.enter_context(tc.tile_pool(name="qk", bufs=1))
        vpool = ctx.enter_context(tc.tile_pool(name="vsb", bufs=1))
        mpool = ctx.enter_context(tc.tile_pool(name="msk", bufs=1))
        opool = ctx.enter_context(tc.tile_pool(name="otn", bufs=1))
        ptpool = ctx.enter_context(tc.tile_pool(name="ptile", bufs=3))
        npool = ctx.enter_context(tc.tile_pool(name="small", bufs=2))
        ospool = ctx.enter_context(tc.tile_pool(name="outsb", bufs=2))
        dpool = ctx.enter_context(tc.tile_pool(name="dscr", bufs=2, space="DRAM"))
        ps = ctx.enter_context(tc.tile_pool(name="ps", bufs=2, space="PSUM"))

        # ---- persistent SBUF tensors -------------------------------------
        wk_sb = wpool.tile([P, TDIN, DG], bf16)
        wv_sb = wpool.tile([P, TDIN, DG], bf16)
        wq_sb = wpool.tile([P, TDIN, DG], bf16)
        for wsb_, wdr_ in ((wk_sb, wk), (wv_sb, wv), (wq_sb, wq)):
            for th_ in range(2):
                nc.sync.dma_start(
                    wsb_[:, th_ * 4 : (th_ + 1) * 4, :],
                    wdr_.ap()[:, th_ * 4 : (th_ + 1) * 4, :],
                )
        qb_sb = cpool.tile([P, NT2], f32)
        kb_sb = cpool.tile([P, NT2], f32)
        vb_sb = cpool.tile([1, DG], bf16)
        nc.sync.dma_start(qb_sb[:], qb.ap())
        nc.sync.dma_start(kb_sb[:], kb.ap())
        nc.sync.dma_start(vb_sb[:], vb.ap())
        ones2 = cpool.tile([1, P], bf16)
        nc.vector.memset(ones2[:], 1.0)

        qT_sb = qkpool.tile([P, NT2, S], bf16)   # q projection, transposed
        kT_sb = qkpool.tile([P, NT2, S], bf16)
        # v blocks: per k-tile, per head: [v(64) | ones] -> 65 cols
        v_sb = vpool.tile([P, KT, HPG * (HD + 1)], bf16)
        nc.vector.memset(
            v_sb[:].rearrange("p s (h x) -> p s h x", h=HPG)[:, :, :, HD : HD + 1],
            1.0,
        )

        # ---- projections (k first, then v, then q) -----------------------
        # q, k: out qT[dout, s] = wq^T(stationary) x q^T(moving) + bias.
        # drains ride ScalarE (idle in this phase); bias folds into the
        # Identity-activation drain as a per-partition scalar.
        # x loads are whole-tensor, th-split into 4 DMAs with 8KB-contiguous
        # rows, except k whose first chunk is small for a fast PE start.
        xpool_cm = tc.tile_pool(name="xin", bufs=2)
        xpool = xpool_cm.__enter__()

        def load_x_chunk(xdram, sc, nm):
            xch = xpool.tile([P, TDIN, 1024], bf16, tag="xch", name=nm)
            for th_ in range(4):
                nc.sync.dma_start(
                    xch[:, th_ * 2 : (th_ + 1) * 2, :],
                    xdram.ap()[
                        :, th_ * 2 : (th_ + 1) * 2, sc * 1024 : (sc + 1) * 1024
                    ],
                )
            return xch

        def qk_chain(xch, wsb, bias_sb, dest, half, s0):
            for dt in range(NT2):
                pj = ps.tile([P, 512], f32, tag="s", name=f"pj_{s0}_{dt}")
                for ktl in range(TDIN):
                    nc.tensor.matmul(
                        pj[:],
                        lhsT=wsb[:, ktl, dt * P : (dt + 1) * P],
                        rhs=xch[:, ktl, half * 512 : (half + 1) * 512],
                        start=(ktl == 0),
                        stop=(ktl == TDIN - 1),
                    )
                nc.scalar.activation(
                    dest[:, dt, s0 : s0 + 512],
                    pj[:],
                    IDENT,
                    bias=bias_sb[:, dt : dt + 1],
                    scale=1.0,
                )

        # k: two 1024-col chunks
        for sc in range(2):
            xck = load_x_chunk(xk, sc, f"xk{sc}")
            for half in range(2):
                qk_chain(xck, wk_sb, kb_sb, kT_sb, half, sc * 1024 + half * 512)

        # v: natural layout [s, dout] + bias, drained per-head with ones col
        for sc in range(2):
            xcv = load_x_chunk(xv, sc, f"xv{sc}")
            for m in range(1024 // P):
                pv = ps.tile([P, DG], f32, tag="s", name=f"pv_{sc}_{m}")
                for ktl in range(TDIN):
                    nc.tensor.matmul(
                        pv[:],
                        lhsT=xcv[:, ktl, m * P : (m + 1) * P],
                        rhs=wv_sb[:, ktl, :],
                        start=(ktl == 0),
                        stop=False,
                    )
                nc.tensor.matmul(
                    pv[:],
                    lhsT=ones2[0:1, :],
                    rhs=vb_sb[:],
                    start=False,
                    stop=True,
                )
                nc.scalar.activation(
                    v_sb[:, sc * (1024 // P) + m, :].rearrange(
                        "p (h x) -> p h x", h=HPG
                    )[:, :, 0:HD],
                    pv[:].rearrange("p (h x) -> p h x", h=HPG),
                    COPY,
                )

        # mask qc0-halves for kt 0..7 land before the q projection so the
        # first attention iterations never wait on them.
        mask_sb = mpool.tile([P, KT, S], bf16)
        mk_r = mk.ap().rearrange("(t p) q -> p t q", p=P)
        xcq0 = load_x_chunk(xq, 0, "xq0")
        for kt in range(8):
            nc.sync.dma_start(mask_sb[:, kt, 0:QW], mk_r[:, kt, 0:QW])
        xcq1 = load_x_chunk(xq, 1, "xq1")
        for half in range(2):
            qk_chain(xcq0, wq_sb, qb_sb, qT_sb, half, half * 512)
        for kt in range(8, KT):
            nc.sync.dma_start(mask_sb[:, kt, 0:QW], mk_r[:, kt, 0:QW])
        for half in range(2):
            qk_chain(xcq1, wq_sb, qb_sb, qT_sb, half, 1024 + half * 512)
        xpool_cm.__exit__(None, None, None)
        for kt in range(KT):
            nc.sync.dma_start(mask_sb[:, kt, QW:S], mk_r[:, kt, QW:S])
        wo_sb = wpool.tile([P, NPAIR, D], bf16)
        nc.sync.dma_start(wo_sb[:], wo.ap())

        # ---- attention ---------------------------------------------------
        # otn2: pair p holds head 2p (partitions 0..63) and head 2p+1
        # (partitions 64..127, via a partition-shift DMA after the norm).
        otn2 = opool.tile([P, NPAIR, S], bf16)

        # The softmax norm of head block i is pipelined into block i+1:
        # its reciprocal + broadcast-bounce start at the top of the next
        # block, and the final multiply lands after 2 mask-muls so the DVE
        # queue never stalls the exp->mul->PV chain.
        pending = None  # (o_ps, pair, parity, q0, tagid)

        def norm_start(pend):
            o_ps, pair, parity, q0, tagid = pend
            rec65 = npool.tile([HD + 1, QW], f32, tag="rec", name=f"rec_{tagid}")
            nc.vector.reciprocal_approx_fast(out=rec65[:], in_=o_ps[:])
            scr = dpool.tile([1, QW], f32, tag="scr", name=f"scr_{tagid}")
            nc.sync.dma_start(scr[:], rec65[HD : HD + 1, :])
            rb = npool.tile([HD, QW], f32, tag="rb", name=f"rb_{tagid}")
            nc.sync.dma_start(rb[:], scr[:].to_broadcast((HD, QW)))
            return rb

        def norm_finish(pend, rb):
            o_ps, pair, parity, q0, tagid = pend
            if parity == 0:
                nc.vector.tensor_mul(
                    otn2[0:HD, pair, q0 : q0 + QW], o_ps[0:HD, :], rb[:]
                )
            else:
                otmp = npool.tile([HD, QW], bf16, tag="otmp", name=f"ot_{tagid}")
                nc.vector.tensor_mul(otmp[:], o_ps[0:HD, :], rb[:])
                nc.sync.dma_start(otn2[HD:P, pair, q0 : q0 + QW], otmp[:])

        for qc in range(QC):
            q0 = qc * QW
            for h in (0, 1, 3, 2):  # even head last: its norm writes otn2
                t, po = h // 2, (h % 2) * HD  # directly (no shift DMA)
                pair = h // 2

                def s_mm(kt):
                    sp = ps.tile(
                        [P, QW], f32, tag="s", name=f"s_{qc}_{h}_{kt}"
                    )
                    for hf in range(2):
                        nc.tensor.matmul(
                            sp[:, hf * 512 : (hf + 1) * 512],
                            lhsT=kT_sb[po : po + HD, t, kt * P : (kt + 1) * P],
                            rhs=qT_sb[
                                po : po + HD,
                                t,
                                q0 + hf * 512 : q0 + (hf + 1) * 512,
                            ],
                            start=True,
                            stop=True,
                        )
                    return sp

                o_ps = ps.tile([HD + 1, QW], f32, tag="o", name=f"o_{qc}_{h}")
                rb = norm_start(pending) if pending is not None else None
                sps = s_mm(0)
                for kt in range(KT):
                    nxt = s_mm(kt + 1) if kt + 1 < KT else None
                    pt = ptpool.tile(
                        [P, QW], bf16, tag="pt", name=f"pt_{qc}_{h}_{kt}"
                    )
                    nc.scalar.activation(pt[:], sps[:], EXP, scale=0.125)
                    nc.vector.tensor_mul(
                        pt[:], pt[:], mask_sb[:, kt, q0 : q0 + QW]
                    )
                    if kt == 2 and pending is not None:
                        norm_finish(pending, rb)
                    for hf in range(2):
                        nc.tensor.matmul(
                            o_ps[:, hf * 512 : (hf + 1) * 512],
                            lhsT=v_sb[:, kt, h * 65 : (h + 1) * 65],
                            rhs=pt[:, hf * 512 : (hf + 1) * 512],
                            start=(kt == 0),
                            stop=(kt == KT - 1),
                        )
                    sps = nxt

                pending = (o_ps, pair, h % 2, q0, f"{qc}_{h}")

        rb = norm_start(pending)
        norm_finish(pending, rb)

        # ---- output projection (tail): pairs stacked on partitions ------
        for st in range(S // P):
            op_ps = ps.tile([P, D], f32, tag="o", name=f"op_{st}")
            for nch in range(2):
                for pr in range(NPAIR):
                    nc.tensor.matmul(
                        op_ps[:, nch * 512 : (nch + 1) * 512],
                        lhsT=otn2[:, pr, st * P : (st + 1) * P],
                        rhs=wo_sb[:, pr, nch * 512 : (nch + 1) * 512],
                        start=(pr == 0),
                        stop=(pr == NPAIR - 1),
                    )
            osb2 = ospool.tile([P, D], bf16, tag="outsb", name=f"outsb_{st}")
            if st % 2 == 0:
                nc.scalar.activation(osb2[:], op_ps[:], COPY)
            else:
                nc.vector.tensor_copy(osb2[:], op_ps[:])
            nc.sync.dma_start(out.ap()[st * P : (st + 1) * P, :], osb2[:])

    nc.compile()
    return nc


@functools.lru_cache(maxsize=1)
def _graph():
    return build_graph()


def make_in_maps(
    query, key, value, mask,
    wq_kernel, wq_bias, wk_kernel, wk_bias,
    wv_kernel, wv_bias, wo_kernel, wo_bias,
):
    q = np.asarray(query, np.float32)
    k = np.asarray(key, np.float32)
    v = np.asarray(value, np.float32)
    mask = np.asarray(mask)
    wqk = np.asarray(wq_kernel, np.float32)
    wkk = np.asarray(wk_kernel, np.float32)
    wvk = np.asarray(wv_kernel, np.float32)
    wok = np.asarray(wo_kernel, np.float32)

    def tile_x(a):  # [S, D] -> [P, TDIN, S] pre-tiled transpose
        return np.ascontiguousarray(
            a.T.reshape(TDIN, P, S).transpose(1, 0, 2)
        ).astype(BF16)

    xt = [[tile_x(x[b]) for x in (q, k, v)] for b in range(B)]
    mt = [
        np.ascontiguousarray(mask[b].T.astype(np.float32)).astype(BF16)
        for b in range(B)
    ]
    in_maps = []
    for c in range(NCORES):
        b, g = divmod(c, GH)
        cs = slice(g * DG, (g + 1) * DG)
        # wo pair layout: [128, NPAIR, D]; pair p = heads (2p, 2p+1)
        wo_arr = np.ascontiguousarray(
            wok[cs, :].reshape(NPAIR, P, D).transpose(1, 0, 2)
        ).astype(BF16)
        in_maps.append(
            {
                "xq_t": xt[b][0],
                "xk_t": xt[b][1],
                "xv_t": xt[b][2],
                "mask_t": mt[b],
                "wq": np.ascontiguousarray(wqk[:, cs].reshape(TDIN, P, DG).transpose(1, 0, 2)).astype(BF16),
                "wk": np.ascontiguousarray(wkk[:, cs].reshape(TDIN, P, DG).transpose(1, 0, 2)).astype(BF16),
                "wv": np.ascontiguousarray(wvk[:, cs].reshape(TDIN, P, DG).transpose(1, 0, 2)).astype(BF16),
                "wo": wo_arr,
                "qb": np.ascontiguousarray(
                    np.asarray(wq_bias, np.float32)[cs].reshape(NT2, P).T
                ),
                "kb": np.ascontiguousarray(
                    np.asarray(wk_bias, np.float32)[cs].reshape(NT2, P).T
                ),
                "vb": np.asarray(wv_bias, np.float32)[cs].reshape(1, DG).astype(BF16),
            }
        )
    return in_maps


def combine_outputs(results, wo_bias):
    outs = np.stack([np.asarray(r["out"], np.float32) for r in results])
    full = outs.reshape(B, GH, S, D).sum(axis=1)
    return (full + np.asarray(wo_bias, np.float32)[None, None, :]).astype(
        np.float32
    )


def kernel(**inputs):
    from concourse import bass_utils

    nc = _graph()
    in_maps = make_in_maps(**inputs)
    res = bass_utils.run_bass_kernel_spmd(
        nc, in_maps, core_ids=list(range(NCORES))
    )
    return combine_outputs(res.results, inputs["wo_bias"])
